# revision 8
# baseline (speedup 1.0000x reference)
"""Distributed Bass kernel for a 4-layer GAT autoencoder on 8 TRN2 NeuronCores.

Strategy (per sharding hint): nodes sharded across 8 cores (2500/core);
edges co-located with their destination node's core, sorted by destination;
params replicated. Node-level compute (x@W, attention score projections) is
replicated on every core; the edge phase (gather + segment softmax + weighted
scatter) is sharded by destination. Layer outputs are AllGathered in-kernel.

v2 changes vs the first working version:
  - tabA (gathered per-edge source rows [h | s_src]) stored in bf16: 1.5-1.8x
    less gather traffic; one-hot scatter matmuls run at bf16 PE rate.
  - the per-edge s_dst table gather is gone: s_dst for a dst tile is 128 rows
    fetched by a tiny 128-index gather (per-core index data keeps this SPMD);
    the per-edge expansion SD[e,:] = s_dst[dst_e,:] is built on the PE via
    transpose(O_j) + an 8-column matmul, overlapping the big source gather.
  - per-chunk elementwise ops batched into per-tile 3D-AP vector ops; the
    one-hot O for all chunks of a tile is built in one DVE op.
  - per-tile chunk counts NCH_t (max over cores, 4-aligned) instead of one
    global max: ~10% fewer padded edges.
  - BN statistics are per-core partial sums computed during the edge phase
    (2 accumulating matmuls/tile into PSUM) + a [128,2] AllReduce — the
    dedicated full-N stats pass is gone.
  - node-phase DMA batched 4 node tiles per transfer (3D access patterns).
"""

import sys

sys.path.insert(0, "/opt/trn_rl_repo")

import numpy as np

P = 128
M = 8
N = 20000
NPC = N // M  # 2500 nodes per core
NT = (NPC + P - 1) // P  # 20 dst tiles per core
HEADS = 8
NEG = 0.2
BN_EPS = 1e-5
DUMMY = N  # dummy table row for pad edges
NEGBIG = -1.0e30

# layer configs: Fin, C (per-head out), concat?, bn on input?, bf16 row width R2
LAYERS = [
    dict(Fin=64, C=16, concat=True, bn=False, R2=256),
    dict(Fin=128, C=32, concat=False, bn=True, R2=384),
    dict(Fin=32, C=16, concat=True, bn=False, R2=256),
    dict(Fin=128, C=64, concat=False, bn=True, R2=640),
]
NODE_TILES = (N + P - 1) // P  # 157 (last has 32 rows)
GB = 4  # node tiles per batched DMA


def _wrap16(idx):
    """Host int array -> dma_gather index layout [16, n/16] (idx[s*16+p] at [p,s])."""
    n = idx.shape[0]
    assert n % 16 == 0
    w = np.ascontiguousarray(idx.reshape(n // 16, 16).T).astype(np.int16)
    return np.ascontiguousarray(np.tile(w, (8, 1)))  # replicated for the 8 Q7 cores


def _preprocess(edge_index):
    """Partition + sort edges; per-tile chunk counts; per-core gather indices."""
    src = np.concatenate([np.asarray(edge_index[0]), np.arange(N)]).astype(np.int64)
    dst = np.concatenate([np.asarray(edge_index[1]), np.arange(N)]).astype(np.int64)

    per_core = []
    cnts = np.zeros((M, NT), dtype=np.int64)
    for m in range(M):
        sel = (dst // NPC) == m
        s, d = src[sel], dst[sel]
        dloc = d - NPC * m
        order = np.argsort(dloc, kind="stable")
        s, dloc = s[order], dloc[order]
        tiles = []
        for t in range(NT):
            tsel = (dloc // P) == t
            st, dt_ = s[tsel], dloc[tsel] - t * P
            tiles.append((st, dt_))
            cnts[m, t] = st.shape[0]
        per_core.append(tiles)

    # per-tile chunk count: max over cores, rounded up to a multiple of 4
    # (keeps every tile's idx segment 64B-aligned for the dma_gather ucode)
    nch = np.maximum(((cnts.max(axis=0) + P - 1) // P + 3) // 4 * 4, 4)
    NCHT = [int(v) for v in nch]
    EPTT = [v * P for v in NCHT]
    TOT = int(sum(EPTT))

    data = []
    for m in range(M):
        isrc = np.full((TOT,), DUMMY, dtype=np.int64)
        dloc_cols = np.full((P, sum(NCHT)), P - 1, dtype=np.float32)
        eoff = 0
        coff = 0
        for t in range(NT):
            st, dt_ = per_core[m][t]
            c = st.shape[0]
            isrc[eoff : eoff + c] = st
            dl = np.full((EPTT[t],), P - 1, dtype=np.int64)
            dl[:c] = dt_
            # column coff+j, row p  = edge (t, j*128+p)
            dloc_cols[:, coff : coff + NCHT[t]] = dl.reshape(NCHT[t], P).T
            eoff += EPTT[t]
            coff += NCHT[t]
        # per-tile own-row ids for the s_dst fetch: 128 idxs per tile, padded
        # to 512 slots so each tile's idx segment stays 64B-aligned
        isdt = np.full((NT * 512,), DUMMY, dtype=np.int64)
        for t in range(NT):
            cnt = min(P, NPC - t * P)
            rows = NPC * m + t * P + np.arange(cnt)
            isdt[t * 512 : t * 512 + cnt] = rows
        data.append(
            dict(
                idx_src=_wrap16(isrc),
                idx_sdt=_wrap16(isdt),
                dstloc=np.ascontiguousarray(dloc_cols),
            )
        )
    return NCHT, data


def _host_consts(inputs):
    """Fused weights + broadcast biases + misc consts (all replicated)."""
    f32 = np.float32
    c = {}
    c["iotab"] = np.tile(np.arange(P, dtype=f32)[None, :], (P, 1))
    c["ident"] = np.eye(P, dtype=f32)
    c["ones"] = np.ones((P, 1), dtype=f32)
    c["dum_a"] = np.full((1, 640), NEGBIG, dtype=f32)
    c["dum_s"] = np.zeros((1, 64), dtype=f32)

    def fuse(W, a_s, a_d):
        # WW = [W | W@blockdiag(a_src) | W@blockdiag(a_dst)]  -> [Fin, HC+16]
        H, C_ = a_s.shape
        Ws = np.einsum("fhc,hc->fh", W.reshape(-1, H, C_), a_s)
        Wd = np.einsum("fhc,hc->fh", W.reshape(-1, H, C_), a_d)
        return np.concatenate([W, Ws, Wd], axis=1).astype(f32)

    c["ww1"] = fuse(inputs["We1"], inputs["as_e1"], inputs["ad_e1"])
    c["ww2"] = fuse(inputs["We2"], inputs["as_e2"], inputs["ad_e2"])
    c["ww3"] = fuse(inputs["Wd1"], inputs["as_d1"], inputs["ad_d1"])
    c["ww4"] = fuse(inputs["Wd2"], inputs["as_d2"], inputs["ad_d2"])
    c["bb1"] = np.tile(inputs["b_e1"][None, :], (P, 1)).astype(f32)  # [128,128]
    c["bb2"] = np.tile(inputs["b_e2"][None, :], (P, 1)).astype(f32)  # [128,32]
    c["bb3"] = np.tile(inputs["b_d1"][None, :], (P, 1)).astype(f32)  # [128,128]
    c["bb4"] = np.tile(inputs["b_d2"][None, :], (P, 1)).astype(f32)  # [128,64]
    c["bn1g"] = inputs["bn1_g"].astype(f32).reshape(-1, 1)  # [128,1]
    c["bn1b"] = inputs["bn1_b"].astype(f32).reshape(-1, 1)
    c["bn2g"] = inputs["bn2_g"].astype(f32).reshape(-1, 1)
    c["bn2b"] = inputs["bn2_b"].astype(f32).reshape(-1, 1)
    c["xin"] = np.asarray(inputs["x"], dtype=f32)
    return c


def _build(NCHT, repeat_k=1):
    from concourse import bacc, bass, mybir, tile

    f32 = mybir.dt.float32
    bf16 = mybir.dt.bfloat16
    i16 = mybir.dt.int16
    nc = bacc.Bacc(
        "TRN2",
        target_bir_lowering=False,
        debug=False,
        enable_asserts=False,
        num_devices=M,
    )

    EPTT = [v * P for v in NCHT]
    TOT = sum(EPTT)
    TOTC = sum(NCHT)
    EOFF = np.concatenate([[0], np.cumsum(EPTT)]).astype(int)
    COFF = np.concatenate([[0], np.cumsum(NCHT)]).astype(int)

    def din(name, shape, dtype=f32):
        return nc.dram_tensor(name, list(shape), dtype, kind="ExternalInput")

    xin = din("xin", (N, 64))
    idx_src = din("idx_src", (128, TOT // 16), i16)
    idx_sdt = din("idx_sdt", (128, NT * 32), i16)
    dstloc = din("dstloc", (P, TOTC))
    iotab = din("iotab", (P, P))
    ident = din("ident", (P, P))
    ones = din("ones", (P, 1))
    dum_a = din("dum_a", (1, 640))
    dum_s = din("dum_s", (1, 64))
    ww = [din(f"ww{l + 1}", (LAYERS[l]["Fin"], HEADS * LAYERS[l]["C"] + 16)) for l in range(4)]
    bb = [
        din("bb1", (P, 128)),
        din("bb2", (P, 32)),
        din("bb3", (P, 128)),
        din("bb4", (P, 64)),
    ]
    bng = [None, din("bn1g", (128, 1)), None, din("bn2g", (128, 1))]
    bnb = [None, din("bn1b", (128, 1)), None, din("bn2b", (128, 1))]
    out_ext = nc.dram_tensor("out", [NPC, 64], f32, kind="ExternalOutput")

    with tile.TileContext(nc) as tc:
        with (
            tc.tile_pool(name="dram", bufs=1, space="DRAM") as dram,
            tc.tile_pool(name="const", bufs=1) as cpool,
            tc.tile_pool(name="work", bufs=3) as wpool,
            tc.tile_pool(name="gath", bufs=2) as gpool,
            tc.tile_pool(name="psum", bufs=2, space="PSUM") as ppool,
        ):
            # ---- internal DRAM ----
            tabA = [
                dram.tile([N + 1, LAYERS[l]["R2"]], bf16, tag=f"tabA{l}", name=f"tabA{l}")
                for l in range(4)
            ]
            tabS = dram.tile([N + 1, 64], f32, tag="tabS", name="tabS")
            own = [
                dram.tile([NPC, 128], f32, tag="own0", name="own0"),
                dram.tile([NPC, 32], f32, tag="own1", name="own1"),
                dram.tile([NPC, 128], f32, tag="own2", name="own2"),
            ]
            ostat = [
                dram.tile([128, 2], f32, tag="ostat0", name="ostat0"),
                None,
                dram.tile([128, 2], f32, tag="ostat2", name="ostat2"),
            ]

            def fresh_shared(rep):
                sfx = "" if rep == 0 else f"r{rep}"
                xg = [
                    dram.tile([N, 128], f32, tag=f"xg0{sfx}", name=f"xg0{sfx}", addr_space="Shared"),
                    dram.tile([N, 32], f32, tag=f"xg1{sfx}", name=f"xg1{sfx}", addr_space="Shared"),
                    dram.tile([N, 128], f32, tag=f"xg2{sfx}", name=f"xg2{sfx}", addr_space="Shared"),
                ]
                gstat = [
                    dram.tile([128, 2], f32, tag=f"gs0{sfx}", name=f"gs0{sfx}", addr_space="Shared"),
                    None,
                    dram.tile([128, 2], f32, tag=f"gs2{sfx}", name=f"gs2{sfx}", addr_space="Shared"),
                ]
                return xg, gstat

            # ---- consts to SBUF ----
            def load_const(ap, shape, dtype=f32, tag=None):
                t = cpool.tile(list(shape), dtype, tag=tag, name=tag)
                nc.sync.dma_start(out=t[:], in_=ap[:])
                return t

            iotaf_sb = load_const(iotab, (P, P), tag="iotaf")
            ident_sb = load_const(ident, (P, P), tag="ident")
            ones_sb = load_const(ones, (P, 1), tag="ones")
            isrc_sb = load_const(idx_src, (128, TOT // 16), i16, tag="isrc")
            isdt_sb = load_const(idx_sdt, (128, NT * 32), i16, tag="isdt")
            dstloc_sb = load_const(dstloc, (P, TOTC), tag="dstloc")
            ww_sb = [
                load_const(ww[l], (LAYERS[l]["Fin"], HEADS * LAYERS[l]["C"] + 16), tag=f"ww{l}")
                for l in range(4)
            ]
            bb_sb = [
                load_const(bb[0], (P, 128), tag="bb0"),
                load_const(bb[1], (P, 32), tag="bb1"),
                load_const(bb[2], (P, 128), tag="bb2"),
                load_const(bb[3], (P, 64), tag="bb3"),
            ]
            bng_sb = [None, load_const(bng[1], (128, 1), tag="bng1"), None, load_const(bng[3], (128, 1), tag="bng3")]
            bnb_sb = [None, load_const(bnb[1], (128, 1), tag="bnb1"), None, load_const(bnb[3], (128, 1), tag="bnb3")]

            AX = mybir.AxisListType.X
            OP = mybir.AluOpType
            AF = mybir.ActivationFunctionType

            # bf16 copies of iota / identity for edge-phase one-hot + transpose
            iotab_sb = cpool.tile([P, P], bf16, tag="iotabf", name="iotabf")
            nc.vector.tensor_copy(iotab_sb[:], iotaf_sb[:])
            identb_sb = cpool.tile([P, P], bf16, tag="identb", name="identb")
            nc.vector.tensor_copy(identb_sb[:], ident_sb[:])

            # dummy rows for pad-edge gathers (tabA row N: NEGBIG, tabS row N: 0)
            dumf = cpool.tile([1, 640], f32, tag="dumf", name="dumf")
            nc.sync.dma_start(out=dumf[:], in_=dum_a[:])
            dumb = cpool.tile([1, 640], bf16, tag="dumb", name="dumb")
            nc.vector.tensor_copy(dumb[:], dumf[:])
            for l in range(4):
                nc.sync.dma_start(
                    out=tabA[l][N : N + 1, : LAYERS[l]["R2"]], in_=dumb[:1, : LAYERS[l]["R2"]]
                )
            nc.sync.dma_start(out=tabS[N : N + 1, :], in_=dum_s[:1, :])

            # registers holding num_idxs values for dma_gather
            ept_regs = {}
            for v in sorted(set(EPTT) | {128}):
                r = nc.alloc_registers(name=f"ept{v}")
                nc.regs_mov(r, v)
                ept_regs[v] = nc.snap(r, donate=False)

            # ================= node phase =================
            def node_phase(l, src_dram, gstat):
                cfg = LAYERS[l]
                Fin, C, R2 = cfg["Fin"], cfg["C"], cfg["R2"]
                HC = HEADS * C
                scale_off = None
                if cfg["bn"]:
                    sg = wpool.tile([128, 2], f32, tag="sg", name="sg")
                    nc.sync.dma_start(out=sg[:], in_=gstat[l - 1][:])
                    mu = wpool.tile([Fin, 1], f32, tag="mu", name="mu")
                    nc.vector.tensor_scalar(mu[:], sg[:, 0:1], 1.0 / N, None, OP.mult)
                    msq = wpool.tile([Fin, 1], f32, tag="msq", name="msq")
                    nc.vector.tensor_scalar(msq[:], sg[:, 1:2], 1.0 / N, None, OP.mult)
                    var = wpool.tile([Fin, 1], f32, tag="var", name="var")
                    nc.vector.tensor_tensor(var[:], mu[:], mu[:], OP.mult)
                    nc.vector.tensor_tensor(var[:], msq[:], var[:], OP.subtract)
                    nc.vector.tensor_scalar(var[:], var[:], BN_EPS, None, OP.add)
                    sdv = wpool.tile([Fin, 1], f32, tag="sdv", name="sdv")
                    nc.scalar.activation(sdv[:], var[:], AF.Sqrt)
                    rs = wpool.tile([Fin, 1], f32, tag="rs", name="rs")
                    nc.vector.reciprocal(rs[:], sdv[:])
                    bscale = wpool.tile([Fin, 1], f32, tag="bscale", name="bscale")
                    nc.vector.tensor_tensor(bscale[:], rs[:], bng_sb[l][:], OP.mult)
                    boff = wpool.tile([Fin, 1], f32, tag="boff", name="boff")
                    nc.vector.tensor_tensor(boff[:], mu[:], bscale[:], OP.mult)
                    nc.vector.tensor_tensor(boff[:], bnb_sb[l][:], boff[:], OP.subtract)
                    scale_off = (bscale, boff)

                def do_tile(xt_slice, tt4, sd4, g):
                    xtp = ppool.tile([Fin, P], f32, tag="xtp", name="xtp", bufs=2)
                    nc.tensor.transpose(out=xtp[:], in_=xt_slice, identity=ident_sb[:])
                    xts = wpool.tile([Fin, P], f32, tag="xts", name="xts")
                    if scale_off is not None:
                        nc.vector.tensor_scalar(
                            xts[:], xtp[:], scale_off[0][:], scale_off[1][:], OP.mult, OP.add
                        )
                        nc.scalar.activation(xts[:], xts[:], AF.Relu)
                    else:
                        nc.vector.tensor_copy(xts[:], xtp[:])
                    o0 = g * (HC + 8)
                    if HC + 16 <= 512:
                        hp = ppool.tile([P, HC + 16], f32, tag="pmm", name="hp", bufs=2)
                        nc.tensor.matmul(out=hp[:], lhsT=xts[:], rhs=ww_sb[l][:], start=True, stop=True)
                        nc.vector.tensor_copy(tt4[:, o0 : o0 + HC + 8], hp[:, : HC + 8])
                        nc.vector.tensor_copy(sd4[:, g * 8 : g * 8 + 8], hp[:, HC + 8 : HC + 16])
                    else:  # L4: 528 cols -> split 512 + 16
                        hp = ppool.tile([P, 512], f32, tag="pmm", name="hp", bufs=2)
                        hp2 = ppool.tile([P, 16], f32, tag="pmm2", name="hp2", bufs=1)
                        nc.tensor.matmul(out=hp[:], lhsT=xts[:], rhs=ww_sb[l][:, :512], start=True, stop=True)
                        nc.tensor.matmul(out=hp2[:], lhsT=xts[:], rhs=ww_sb[l][:, 512:], start=True, stop=True)
                        nc.vector.tensor_copy(tt4[:, o0 : o0 + 512], hp[:])
                        nc.vector.tensor_copy(tt4[:, o0 + 512 : o0 + 520], hp2[:, 0:8])
                        nc.vector.tensor_copy(sd4[:, g * 8 : g * 8 + 8], hp2[:, 8:16])

                ngrp = NODE_TILES // GB  # 39 full groups; tail tile separate
                for gi in range(ngrp):
                    r0 = gi * GB * P
                    xt4 = wpool.tile([P, GB * Fin], f32, tag="xt4", name="xt4")
                    nc.sync.dma_start(
                        out=xt4[:].rearrange("p (g f) -> p g f", f=Fin),
                        in_=src_dram[r0 : r0 + GB * P, :].rearrange("(g p) f -> p g f", p=P),
                    )
                    tt4 = wpool.tile([P, GB * (HC + 8)], bf16, tag="tt4", name="tt4")
                    sd4 = wpool.tile([P, GB * 8], f32, tag="sd4", name="sd4")
                    for g in range(GB):
                        do_tile(xt4[:, g * Fin : (g + 1) * Fin], tt4, sd4, g)
                    nc.sync.dma_start(
                        out=tabA[l][r0 : r0 + GB * P, : HC + 8].rearrange("(g p) f -> p g f", p=P),
                        in_=tt4[:].rearrange("p (g f) -> p g f", f=HC + 8),
                    )
                    nc.sync.dma_start(
                        out=tabS[r0 : r0 + GB * P, :8].rearrange("(g p) f -> p g f", p=P),
                        in_=sd4[:].rearrange("p (g f) -> p g f", f=8),
                    )
                # tail tile (32 rows)
                r0 = ngrp * GB * P
                cnt = N - r0
                xtt = wpool.tile([P, Fin], f32, tag="xt4t", name="xt4t")
                nc.vector.memset(xtt[:], 0.0)
                nc.sync.dma_start(out=xtt[:cnt, :], in_=src_dram[r0:N, :])
                ttt = wpool.tile([P, HC + 8], bf16, tag="tt4t", name="tt4t")
                sdt_ = wpool.tile([P, 8], f32, tag="sd4t", name="sd4t")
                do_tile(xtt[:, :], ttt, sdt_, 0)
                nc.sync.dma_start(out=tabA[l][r0:N, : HC + 8], in_=ttt[:cnt, :])
                nc.sync.dma_start(out=tabS[r0:N, :8], in_=sdt_[:cnt, :])

            # ================= edge phase =================
            def edge_phase(l, out_dram):
                cfg = LAYERS[l]
                C, R2 = cfg["C"], cfg["R2"]
                HC = HEADS * C
                do_stat = l in (0, 2)
                if do_stat:
                    accS = wpool.tile([HC, 2], f32, tag="accS", name="accS")
                    nc.vector.memset(accS[:], 0.0)
                for t in range(NT):
                    NCH = NCHT[t]
                    EPT = EPTT[t]
                    cnt = min(P, NPC - t * P)
                    # one-hot O for all chunks of this tile (single DVE op)
                    Oall = gpool.tile([P, NCH * P], bf16, tag="Oall", name="Oall")
                    nc.vector.tensor_tensor(
                        Oall[:].rearrange("p (j f) -> p j f", f=P),
                        iotab_sb[:].unsqueeze(1).to_broadcast((P, NCH, P)),
                        dstloc_sb[:, COFF[t] : COFF[t] + NCH].unsqueeze(2).to_broadcast((P, NCH, P)),
                        OP.is_equal,
                    )
                    # own s_dst rows for this tile via a tiny 128-idx gather
                    sdtf = gpool.tile([P, 64], f32, tag="sdtf", name="sdtf")
                    nc.gpsimd.dma_gather(
                        out_ap=sdtf[:].rearrange("p (s r) -> p s r", r=64),
                        in_ap=tabS[:],
                        idxs_ap=isdt_sb[:, t * 32 : t * 32 + 8],
                        num_idxs=128,
                        num_idxs_reg=ept_regs[128],
                        elem_size=64,
                        single_packet=False,
                    )
                    sdtb = wpool.tile([P, 8], bf16, tag="sdtb", name="sdtb")
                    nc.vector.tensor_copy(sdtb[:], sdtf[:, :8])
                    # per-chunk SD = transpose(O_j).T @ sdt  (PE; overlaps gather)
                    psSD = ppool.tile([P, NCH * 8], f32, tag="psSD", name="psSD", bufs=1)
                    for j in range(NCH):
                        psOT = ppool.tile([P, P], bf16, tag="xtp", name="psOT", bufs=2)
                        nc.tensor.transpose(
                            out=psOT[:], in_=Oall[:, j * P : (j + 1) * P], identity=identb_sb[:]
                        )
                        OTs = wpool.tile([P, P], bf16, tag="OTs", name="OTs")
                        nc.vector.tensor_copy(OTs[:], psOT[:])
                        nc.tensor.matmul(
                            out=psSD[:, j * 8 : (j + 1) * 8], lhsT=OTs[:], rhs=sdtb[:],
                            start=True, stop=True,
                        )
                    # gather source rows [h | s_src] (bf16)
                    G = gpool.tile([P, NCH * R2], bf16, tag="G", name="G")
                    nc.gpsimd.dma_gather(
                        out_ap=G[:].rearrange("p (j r) -> p j r", r=R2),
                        in_ap=tabA[l][:],
                        idxs_ap=isrc_sb[:, EOFF[t] // 16 : EOFF[t + 1] // 16],
                        num_idxs=EPT,
                        num_idxs_reg=ept_regs[EPT],
                        elem_size=R2,
                        single_packet=False,
                    )
                    G3 = G[:].rearrange("p (j r) -> p j r", r=R2)
                    # batched e = LeakyReLU(s_src + s_dst); EXS = exp(e) (bf16)
                    EB = wpool.tile([P, NCH * 8], f32, tag="EB", name="EB")
                    nc.vector.tensor_copy(
                        EB[:].rearrange("p (j r) -> p j r", r=8), G3[:, :, HC : HC + 8]
                    )
                    nc.vector.tensor_tensor(EB[:], EB[:], psSD[:], OP.add)
                    EB2 = wpool.tile([P, NCH * 8], f32, tag="EB2", name="EB2")
                    nc.vector.tensor_scalar(EB2[:], EB[:], NEG, None, OP.mult)
                    nc.vector.tensor_tensor(EB[:], EB[:], EB2[:], OP.max)
                    EXS = wpool.tile([P, NCH * 8], bf16, tag="EXS", name="EXS")
                    nc.scalar.activation(EXS[:], EB[:], AF.Exp)
                    # per-chunk weighted scatter
                    if HC + 8 <= 512:
                        psA = ppool.tile([P, HC + 8], f32, tag="pmm", name="psA", bufs=2)
                        psB = None
                    else:
                        psA = ppool.tile([P, 512], f32, tag="pmm", name="psA", bufs=2)
                        psB = ppool.tile([P, 8], f32, tag="pmm2", name="psB", bufs=1)
                    for j in range(NCH):
                        GEX = wpool.tile([P, HC + 8], bf16, tag="GEX", name="GEX")
                        nc.vector.tensor_tensor(
                            GEX[:, :HC].rearrange("p (h c) -> p h c", h=HEADS),
                            G3[:, j, :HC].rearrange("p (h c) -> p h c", h=HEADS),
                            EXS[:, j * 8 : (j + 1) * 8].unsqueeze(2).to_broadcast((P, HEADS, C)),
                            OP.mult,
                        )
                        nc.vector.tensor_copy(GEX[:, HC : HC + 8], EXS[:, j * 8 : (j + 1) * 8])
                        if psB is None:
                            nc.tensor.matmul(
                                out=psA[:], lhsT=Oall[:, j * P : (j + 1) * P], rhs=GEX[:],
                                start=(j == 0), stop=(j == NCH - 1),
                            )
                        else:
                            nc.tensor.matmul(
                                out=psA[:], lhsT=Oall[:, j * P : (j + 1) * P], rhs=GEX[:, :512],
                                start=(j == 0), stop=(j == NCH - 1),
                            )
                            nc.tensor.matmul(
                                out=psB[:], lhsT=Oall[:, j * P : (j + 1) * P], rhs=GEX[:, 512:],
                                start=(j == 0), stop=(j == NCH - 1),
                            )
                    den = psA[:, HC : HC + 8] if psB is None else psB[:]
                    rec = wpool.tile([P, 8], f32, tag="rec", name="rec")
                    nc.vector.tensor_scalar(rec[:], den, 1e-16, None, OP.add)
                    nc.vector.reciprocal(rec[:], rec[:])
                    res = wpool.tile([P, HC], f32, tag="res", name="res")
                    nc.vector.tensor_tensor(
                        res[:].rearrange("p (h c) -> p h c", h=HEADS),
                        psA[:, :HC].rearrange("p (h c) -> p h c", h=HEADS),
                        rec[:].unsqueeze(2).to_broadcast((P, HEADS, C)),
                        OP.mult,
                    )
                    if cfg["concat"]:
                        nc.vector.tensor_tensor(res[:], res[:], bb_sb[l][:], OP.add)
                        nc.sync.dma_start(
                            out=out_dram[t * P : t * P + cnt, :], in_=res[:cnt, :]
                        )
                        if do_stat:
                            sq = wpool.tile([P, HC], f32, tag="sq", name="sq")
                            nc.scalar.square(sq[:], res[:])
                            psS1 = ppool.tile([HC, 1], f32, tag="psS", name="psS1", bufs=2)
                            nc.tensor.matmul(
                                out=psS1[:], lhsT=res[:cnt, :], rhs=ones_sb[:cnt, :],
                                start=True, stop=True,
                            )
                            nc.vector.tensor_tensor(accS[:, 0:1], accS[:, 0:1], psS1[:], OP.add)
                            psS2 = ppool.tile([HC, 1], f32, tag="psS", name="psS2", bufs=2)
                            nc.tensor.matmul(
                                out=psS2[:], lhsT=sq[:cnt, :], rhs=ones_sb[:cnt, :],
                                start=True, stop=True,
                            )
                            nc.vector.tensor_tensor(accS[:, 1:2], accS[:, 1:2], psS2[:], OP.add)
                    else:
                        red = wpool.tile([P, C], f32, tag="red", name="red")
                        nc.vector.tensor_reduce(
                            red[:],
                            res[:].rearrange("p (h c) -> p c h", h=HEADS),
                            AX,
                            OP.add,
                        )
                        nc.vector.tensor_scalar(red[:], red[:], 1.0 / HEADS, None, OP.mult)
                        nc.vector.tensor_tensor(red[:], red[:], bb_sb[l][:, :C], OP.add)
                        nc.sync.dma_start(
                            out=out_dram[t * P : t * P + cnt, :], in_=red[:cnt, :]
                        )
                if do_stat:
                    nc.sync.dma_start(out=ostat[l][:], in_=accS[:])

            # ================= full pipeline =================
            for _rep in range(repeat_k):
                xg, gstat = fresh_shared(_rep)
                srcs = [xin, xg[0], xg[1], xg[2]]
                outs = [own[0], own[1], own[2], out_ext]
                for l in range(4):
                    node_phase(l, srcs[l], gstat)
                    edge_phase(l, outs[l])
                    if l < 3:
                        if l in (0, 2):
                            nc.gpsimd.collective_compute(
                                "AllReduce",
                                mybir.AluOpType.add,
                                replica_groups=[list(range(M))],
                                ins=[ostat[l].opt()],
                                outs=[gstat[l].opt()],
                            )
                        nc.gpsimd.collective_compute(
                            "AllGather",
                            mybir.AluOpType.bypass,
                            replica_groups=[list(range(M))],
                            ins=[own[l].opt()],
                            outs=[xg[l].opt()],
                        )
    if not nc.is_finalized():
        nc.finalize()
    return nc


def _pjrt_exec(nc, in_maps, time_reps=0):
    """Mirror of bass2jax.run_bass_via_pjrt multi-core path, holding the jitted
    executable so repeated executions can be wall-timed."""
    import time as _t
    import jax
    from jax.experimental.shard_map import shard_map
    from jax.sharding import Mesh, PartitionSpec
    from concourse import bass2jax as B, mybir as mb

    B.install_neuronx_cc_hook()
    n_cores = len(in_maps)
    partition_name = nc.partition_id_tensor.name if nc.partition_id_tensor else None
    in_names, out_names, out_avals, zero_outs = [], [], [], []
    for alloc in nc.m.functions[0].allocations:
        if not isinstance(alloc, mb.MemoryLocationSet):
            continue
        name = alloc.memorylocations[0].name
        if alloc.kind == "ExternalInput":
            if name != partition_name:
                in_names.append(name)
        elif alloc.kind == "ExternalOutput":
            out_names.append(name)
            shape = tuple(alloc.tensor_shape)
            dtype = mb.dt.np(alloc.dtype)
            out_avals.append(jax.core.ShapedArray(shape, dtype))
            zero_outs.append(np.zeros(shape, dtype))
    n_params = len(in_names)
    n_outs = len(out_avals)
    in_names.extend(out_names)
    if partition_name is not None:
        in_names.append(partition_name)
    donate = tuple(range(n_params, n_params + n_outs))

    def _body(*args):
        operands = list(args)
        if partition_name is not None:
            operands.append(B.partition_id_tensor())
        outs = B._bass_exec_p.bind(
            *operands,
            out_avals=tuple(out_avals),
            in_names=tuple(in_names),
            out_names=tuple(out_names),
            lowering_input_output_aliases=(),
            sim_require_finite=True,
            sim_require_nnan=True,
            nc=nc,
        )
        return tuple(outs)

    devices = jax.devices()[:n_cores]
    mesh = Mesh(np.asarray(devices), ("core",))
    in_specs = (PartitionSpec("core"),) * (n_params + n_outs)
    out_specs = (PartitionSpec("core"),) * len(out_names)
    sharded = jax.jit(
        shard_map(_body, mesh=mesh, in_specs=in_specs, out_specs=out_specs,
                  check_rep=False),
        donate_argnums=donate, keep_unused=True,
    )
    per_core = [[np.asarray(m_[nm]) for nm in in_names[:n_params]] for m_ in in_maps]
    concat_in = [
        np.concatenate([per_core[c][i] for c in range(n_cores)], axis=0)
        for i in range(n_params)
    ]
    from jax.sharding import NamedSharding
    shard = NamedSharding(mesh, PartitionSpec("core"))
    concat_in = [jax.device_put(a, shard) for a in concat_in]
    jax.block_until_ready(concat_in)

    def once():
        cz = [jax.device_put(np.zeros((n_cores * z.shape[0], *z.shape[1:]), z.dtype), shard)
              for z in zero_outs]
        jax.block_until_ready(cz)
        t0 = _t.perf_counter()
        out_arrs = sharded(*concat_in, *cz)
        jax.block_until_ready(out_arrs)
        return _t.perf_counter() - t0, out_arrs

    _, out_arrs = once()  # compile + first run
    times = []
    for _ in range(time_reps):
        dt, out_arrs = once()
        times.append(dt)
    res = [
        {nm: np.asarray(out_arrs[i]).reshape(n_cores, *out_avals[i].shape)[c]
         for i, nm in enumerate(out_names)}
        for c in range(n_cores)
    ]
    return res, (min(times) if times else None)


def _run(inputs, trace=False, time_reps=0, repeat_k=1):
    NCHT, edata = _preprocess(np.asarray(inputs["edge_index"]))
    consts = _host_consts(inputs)
    nc = _build(NCHT, repeat_k=repeat_k)

    in_maps = []
    for m in range(M):
        d = dict(consts)
        d.update(edata[m])
        in_maps.append(d)

    if time_reps > 0:
        results, best_s = _pjrt_exec(nc, in_maps, time_reps=time_reps)
    else:
        from concourse.bass_utils import run_bass_kernel_spmd

        res = run_bass_kernel_spmd(nc, in_maps, core_ids=list(range(M)))
        results, best_s = res.results, None
    outs = [np.asarray(results[m]["out"]) for m in range(M)]
    full = np.concatenate(outs, axis=0).astype(np.float32)
    return full, (None if best_s is None else int(best_s * 1e9))


def kernel(**inputs):
    out, _ = _run(inputs, trace=False)
    return out


# revision 9
# speedup vs baseline: 1.4390x; 1.4390x over previous
"""Distributed Bass kernel for a 4-layer GAT autoencoder on 8 TRN2 NeuronCores.

Strategy (per sharding hint): nodes sharded across 8 cores (2500/core);
edges co-located with their destination node's core, sorted by destination;
params replicated.

v3 layout: the node phase is SHARDED — each core transforms only its own
2500 rows (inputs pre-sliced per core host-side, so all node-phase DMA uses
static local offsets), then the compact bf16 per-node tables [h | s_src]
are AllGathered so the edge phase can gather any source row. s_dst stays
local (a [2500,8] buffer). Pad edges carry an out-of-range dstloc (999), so
their one-hot column is all zeros and they contribute nothing to the
scatter or the softmax denominator — no NEGBIG dummy-row machinery.

Edge phase per dst tile: one-hot O for all chunks in one DVE op; a big bf16
dma_gather of source rows; per-edge s_dst via PE transpose(O_j) + an
8-column matmul; batched LeakyReLU/exp; per-chunk h*ex one-hot scatter
matmuls accumulating numerator and denominator in PSUM. BN statistics are
per-core partial sums (closed-group matmuls + SBUF accumulate) + a [128,2]
AllReduce; there is no separate stats pass.
"""

import sys

sys.path.insert(0, "/opt/trn_rl_repo")

import numpy as np

P = 128
M = 8
N = 20000
NPC = N // M  # 2500 nodes per core
NT = (NPC + P - 1) // P  # 20 dst tiles per core
HEADS = 8
NEG = 0.2
BN_EPS = 1e-5
PADDST = 999.0  # out-of-range dst slot for pad edges -> zero one-hot column

# layer configs: Fin, C (per-head out), concat?, bn on input?, bf16 row width R2
LAYERS = [
    dict(Fin=64, C=16, concat=True, bn=False, R2=256),
    dict(Fin=128, C=32, concat=False, bn=True, R2=384),
    dict(Fin=32, C=16, concat=True, bn=False, R2=256),
    dict(Fin=128, C=64, concat=False, bn=True, R2=640),
]
OWNW = [128, 32, 128, 64]  # own[l] row widths


def _wrap16(idx):
    """Host int array -> dma_gather index layout [16, n/16] (idx[s*16+p] at [p,s])."""
    n = idx.shape[0]
    assert n % 16 == 0
    w = np.ascontiguousarray(idx.reshape(n // 16, 16).T).astype(np.int16)
    return np.ascontiguousarray(np.tile(w, (8, 1)))  # replicated for the 8 Q7 cores


def _preprocess(edge_index):
    """Partition + sort edges; per-tile chunk counts; per-core gather indices."""
    src = np.concatenate([np.asarray(edge_index[0]), np.arange(N)]).astype(np.int64)
    dst = np.concatenate([np.asarray(edge_index[1]), np.arange(N)]).astype(np.int64)

    per_core = []
    cnts = np.zeros((M, NT), dtype=np.int64)
    for m in range(M):
        sel = (dst // NPC) == m
        s, d = src[sel], dst[sel]
        dloc = d - NPC * m
        order = np.argsort(dloc, kind="stable")
        s, dloc = s[order], dloc[order]
        tiles = []
        for t in range(NT):
            tsel = (dloc // P) == t
            st, dt_ = s[tsel], dloc[tsel] - t * P
            tiles.append((st, dt_))
            cnts[m, t] = st.shape[0]
        per_core.append(tiles)

    # per-tile chunk count: max over cores, rounded up to a multiple of 4
    # (keeps every tile's idx segment 64B-aligned for the dma_gather ucode)
    nch = np.maximum(((cnts.max(axis=0) + P - 1) // P + 3) // 4 * 4, 4)
    NCHT = [int(v) for v in nch]
    EPTT = [v * P for v in NCHT]
    TOT = int(sum(EPTT))

    data = []
    for m in range(M):
        isrc = np.zeros((TOT,), dtype=np.int64)  # pad edges gather row 0
        dloc_cols = np.full((P, sum(NCHT)), PADDST, dtype=np.float32)
        eoff = 0
        coff = 0
        for t in range(NT):
            st, dt_ = per_core[m][t]
            c = st.shape[0]
            isrc[eoff : eoff + c] = st
            dl = np.full((EPTT[t],), PADDST, dtype=np.float64)
            dl[:c] = dt_
            # column coff+j, row p  = edge (t, j*128+p)
            dloc_cols[:, coff : coff + NCHT[t]] = dl.reshape(NCHT[t], P).T
            eoff += EPTT[t]
            coff += NCHT[t]
        data.append(
            dict(idx_src=_wrap16(isrc), dstloc=np.ascontiguousarray(dloc_cols))
        )
    return NCHT, data


def _host_consts(inputs):
    """Fused weights + broadcast biases + misc consts."""
    f32 = np.float32
    c = {}
    c["iotab"] = np.tile(np.arange(P, dtype=f32)[None, :], (P, 1))
    c["ident"] = np.eye(P, dtype=f32)
    c["ones"] = np.ones((P, 1), dtype=f32)

    def fuse(W, a_s, a_d):
        # WW = [W | W@blockdiag(a_src) | W@blockdiag(a_dst)]  -> [Fin, HC+16]
        H, C_ = a_s.shape
        Ws = np.einsum("fhc,hc->fh", W.reshape(-1, H, C_), a_s)
        Wd = np.einsum("fhc,hc->fh", W.reshape(-1, H, C_), a_d)
        return np.concatenate([W, Ws, Wd], axis=1).astype(f32)

    c["ww1"] = fuse(inputs["We1"], inputs["as_e1"], inputs["ad_e1"])
    c["ww2"] = fuse(inputs["We2"], inputs["as_e2"], inputs["ad_e2"])
    c["ww3"] = fuse(inputs["Wd1"], inputs["as_d1"], inputs["ad_d1"])
    c["ww4"] = fuse(inputs["Wd2"], inputs["as_d2"], inputs["ad_d2"])
    c["bb1"] = np.tile(inputs["b_e1"][None, :], (P, 1)).astype(f32)  # [128,128]
    c["bb2"] = np.tile(inputs["b_e2"][None, :], (P, 1)).astype(f32)  # [128,32]
    c["bb3"] = np.tile(inputs["b_d1"][None, :], (P, 1)).astype(f32)  # [128,128]
    c["bb4"] = np.tile(inputs["b_d2"][None, :], (P, 1)).astype(f32)  # [128,64]
    c["bn1g"] = inputs["bn1_g"].astype(f32).reshape(-1, 1)  # [128,1]
    c["bn1b"] = inputs["bn1_b"].astype(f32).reshape(-1, 1)
    c["bn2g"] = inputs["bn2_g"].astype(f32).reshape(-1, 1)
    c["bn2b"] = inputs["bn2_b"].astype(f32).reshape(-1, 1)
    return c


def _build(NCHT, repeat_k=1):
    from concourse import bacc, bass, mybir, tile

    f32 = mybir.dt.float32
    bf16 = mybir.dt.bfloat16
    i16 = mybir.dt.int16
    nc = bacc.Bacc(
        "TRN2",
        target_bir_lowering=False,
        debug=False,
        enable_asserts=False,
        num_devices=M,
    )

    EPTT = [v * P for v in NCHT]
    TOT = sum(EPTT)
    TOTC = sum(NCHT)
    EOFF = np.concatenate([[0], np.cumsum(EPTT)]).astype(int)
    COFF = np.concatenate([[0], np.cumsum(NCHT)]).astype(int)

    def din(name, shape, dtype=f32):
        return nc.dram_tensor(name, list(shape), dtype, kind="ExternalInput")

    xin = din("xin", (NPC, 64))  # per-core slice of x
    idx_src = din("idx_src", (128, TOT // 16), i16)
    dstloc = din("dstloc", (P, TOTC))
    iotab = din("iotab", (P, P))
    ident = din("ident", (P, P))
    ones = din("ones", (P, 1))
    ww = [din(f"ww{l + 1}", (LAYERS[l]["Fin"], HEADS * LAYERS[l]["C"] + 16)) for l in range(4)]
    bb = [
        din("bb1", (P, 128)),
        din("bb2", (P, 32)),
        din("bb3", (P, 128)),
        din("bb4", (P, 64)),
    ]
    bng = [None, din("bn1g", (128, 1)), None, din("bn2g", (128, 1))]
    bnb = [None, din("bn1b", (128, 1)), None, din("bn2b", (128, 1))]
    out_ext = nc.dram_tensor("out", [NPC, 64], f32, kind="ExternalOutput")

    with tile.TileContext(nc) as tc:
        with (
            tc.tile_pool(name="dram", bufs=1, space="DRAM") as dram,
            tc.tile_pool(name="const", bufs=1) as cpool,
            tc.tile_pool(name="work", bufs=3) as wpool,
            tc.tile_pool(name="gath", bufs=2) as gpool,
            tc.tile_pool(name="psum", bufs=2, space="PSUM") as ppool,
        ):
            # ---- internal DRAM (local) ----
            tabL = [
                dram.tile([NPC, LAYERS[l]["R2"]], bf16, tag=f"tabL{l}", name=f"tabL{l}")
                for l in range(4)
            ]
            sdo = dram.tile([NPC, 8], f32, tag="sdo", name="sdo")
            own = [
                dram.tile([NPC, OWNW[l]], f32, tag=f"own{l}", name=f"own{l}")
                for l in range(3)
            ]
            ostat = [
                dram.tile([128, 2], f32, tag="ostat0", name="ostat0"),
                None,
                dram.tile([128, 2], f32, tag="ostat2", name="ostat2"),
            ]

            def fresh_shared(rep):
                sfx = "" if rep == 0 else f"r{rep}"
                tabA = [
                    dram.tile([N, LAYERS[l]["R2"]], bf16, tag=f"tabA{l}{sfx}",
                              name=f"tabA{l}{sfx}", addr_space="Shared")
                    for l in range(4)
                ]
                gstat = [
                    dram.tile([128, 2], f32, tag=f"gs0{sfx}", name=f"gs0{sfx}", addr_space="Shared"),
                    None,
                    dram.tile([128, 2], f32, tag=f"gs2{sfx}", name=f"gs2{sfx}", addr_space="Shared"),
                ]
                return tabA, gstat

            # ---- consts to SBUF ----
            def load_const(ap, shape, dtype=f32, tag=None):
                t = cpool.tile(list(shape), dtype, tag=tag, name=tag)
                nc.sync.dma_start(out=t[:], in_=ap[:])
                return t

            iotaf_sb = load_const(iotab, (P, P), tag="iotaf")
            ident_sb = load_const(ident, (P, P), tag="ident")
            ones_sb = load_const(ones, (P, 1), tag="ones")
            isrc_sb = load_const(idx_src, (128, TOT // 16), i16, tag="isrc")
            dstloc_sb = load_const(dstloc, (P, TOTC), tag="dstloc")
            ww_sb = [
                load_const(ww[l], (LAYERS[l]["Fin"], HEADS * LAYERS[l]["C"] + 16), tag=f"ww{l}")
                for l in range(4)
            ]
            bb_sb = [
                load_const(bb[0], (P, 128), tag="bb0"),
                load_const(bb[1], (P, 32), tag="bb1"),
                load_const(bb[2], (P, 128), tag="bb2"),
                load_const(bb[3], (P, 64), tag="bb3"),
            ]
            bng_sb = [None, load_const(bng[1], (128, 1), tag="bng1"), None, load_const(bng[3], (128, 1), tag="bng3")]
            bnb_sb = [None, load_const(bnb[1], (128, 1), tag="bnb1"), None, load_const(bnb[3], (128, 1), tag="bnb3")]

            AX = mybir.AxisListType.X
            OP = mybir.AluOpType
            AF = mybir.ActivationFunctionType

            iotab_sb = cpool.tile([P, P], bf16, tag="iotabf", name="iotabf")
            nc.vector.tensor_copy(iotab_sb[:], iotaf_sb[:])
            identb_sb = cpool.tile([P, P], bf16, tag="identb", name="identb")
            nc.vector.tensor_copy(identb_sb[:], ident_sb[:])

            # registers holding num_idxs values for dma_gather
            ept_regs = {}
            for v in sorted(set(EPTT)):
                r = nc.alloc_registers(name=f"ept{v}")
                nc.regs_mov(r, v)
                ept_regs[v] = nc.snap(r, donate=False)

            # ============ node phase (own 2500 rows only) ============
            def node_phase(l, src_dram, gstat):
                cfg = LAYERS[l]
                Fin, C, R2 = cfg["Fin"], cfg["C"], cfg["R2"]
                HC = HEADS * C
                scale_off = None
                if cfg["bn"]:
                    sg = wpool.tile([128, 2], f32, tag="sg", name="sg")
                    nc.sync.dma_start(out=sg[:], in_=gstat[l - 1][:])
                    mu = wpool.tile([Fin, 1], f32, tag="mu", name="mu")
                    nc.vector.tensor_scalar(mu[:], sg[:, 0:1], 1.0 / N, None, OP.mult)
                    msq = wpool.tile([Fin, 1], f32, tag="msq", name="msq")
                    nc.vector.tensor_scalar(msq[:], sg[:, 1:2], 1.0 / N, None, OP.mult)
                    var = wpool.tile([Fin, 1], f32, tag="var", name="var")
                    nc.vector.tensor_tensor(var[:], mu[:], mu[:], OP.mult)
                    nc.vector.tensor_tensor(var[:], msq[:], var[:], OP.subtract)
                    nc.vector.tensor_scalar(var[:], var[:], BN_EPS, None, OP.add)
                    sdv = wpool.tile([Fin, 1], f32, tag="sdv", name="sdv")
                    nc.scalar.activation(sdv[:], var[:], AF.Sqrt)
                    rs = wpool.tile([Fin, 1], f32, tag="rs", name="rs")
                    nc.vector.reciprocal(rs[:], sdv[:])
                    bscale = wpool.tile([Fin, 1], f32, tag="bscale", name="bscale")
                    nc.vector.tensor_tensor(bscale[:], rs[:], bng_sb[l][:], OP.mult)
                    boff = wpool.tile([Fin, 1], f32, tag="boff", name="boff")
                    nc.vector.tensor_tensor(boff[:], mu[:], bscale[:], OP.mult)
                    nc.vector.tensor_tensor(boff[:], bnb_sb[l][:], boff[:], OP.subtract)
                    scale_off = (bscale, boff)

                for t in range(NT):
                    cnt = min(P, NPC - t * P)
                    xt = wpool.tile([P, Fin], f32, tag="xt", name="xt")
                    if cnt < P:
                        nc.vector.memset(xt[:], 0.0)
                    nc.sync.dma_start(out=xt[:cnt, :], in_=src_dram[t * P : t * P + cnt, :Fin])
                    xtp = ppool.tile([Fin, P], f32, tag="xtp", name="xtp", bufs=2)
                    nc.tensor.transpose(out=xtp[:], in_=xt[:], identity=ident_sb[:])
                    xts = wpool.tile([Fin, P], f32, tag="xts", name="xts")
                    if scale_off is not None:
                        nc.vector.tensor_scalar(
                            xts[:], xtp[:], scale_off[0][:], scale_off[1][:], OP.mult, OP.add
                        )
                        nc.scalar.activation(xts[:], xts[:], AF.Relu)
                    else:
                        nc.vector.tensor_copy(xts[:], xtp[:])
                    tt = wpool.tile([P, HC + 8], bf16, tag="tt", name="tt")
                    sd = wpool.tile([P, 8], f32, tag="sd", name="sd")
                    if HC + 16 <= 512:
                        hp = ppool.tile([P, HC + 16], f32, tag="pmm", name="hp", bufs=2)
                        nc.tensor.matmul(out=hp[:], lhsT=xts[:], rhs=ww_sb[l][:], start=True, stop=True)
                        nc.vector.tensor_copy(tt[:], hp[:, : HC + 8])
                        nc.vector.tensor_copy(sd[:], hp[:, HC + 8 : HC + 16])
                    else:  # L4: 528 cols -> split 512 + 16
                        hp = ppool.tile([P, 512], f32, tag="pmm", name="hp", bufs=2)
                        hp2 = ppool.tile([P, 16], f32, tag="pmm2", name="hp2", bufs=1)
                        nc.tensor.matmul(out=hp[:], lhsT=xts[:], rhs=ww_sb[l][:, :512], start=True, stop=True)
                        nc.tensor.matmul(out=hp2[:], lhsT=xts[:], rhs=ww_sb[l][:, 512:], start=True, stop=True)
                        nc.vector.tensor_copy(tt[:, :512], hp[:])
                        nc.vector.tensor_copy(tt[:, 512:520], hp2[:, 0:8])
                        nc.vector.tensor_copy(sd[:], hp2[:, 8:16])
                    nc.sync.dma_start(
                        out=tabL[l][t * P : t * P + cnt, : HC + 8], in_=tt[:cnt, :]
                    )
                    nc.sync.dma_start(out=sdo[t * P : t * P + cnt, :], in_=sd[:cnt, :])

            # ============ edge phase (own dst tiles) ============
            def edge_phase(l, tabA, out_dram):
                cfg = LAYERS[l]
                C, R2 = cfg["C"], cfg["R2"]
                HC = HEADS * C
                do_stat = l in (0, 2)
                if do_stat:
                    accS = wpool.tile([HC, 2], f32, tag="accS", name="accS")
                    nc.vector.memset(accS[:], 0.0)
                for t in range(NT):
                    NCH = NCHT[t]
                    EPT = EPTT[t]
                    cnt = min(P, NPC - t * P)
                    # one-hot O for all chunks of this tile (single DVE op);
                    # pad edges have dstloc=999 -> all-zero column
                    Oall = gpool.tile([P, NCH * P], bf16, tag="Oall", name="Oall")
                    nc.vector.tensor_tensor(
                        Oall[:].rearrange("p (j f) -> p j f", f=P),
                        iotab_sb[:].unsqueeze(1).to_broadcast((P, NCH, P)),
                        dstloc_sb[:, COFF[t] : COFF[t] + NCH].unsqueeze(2).to_broadcast((P, NCH, P)),
                        OP.is_equal,
                    )
                    # own s_dst rows for this tile (local, static offset)
                    sdtf = wpool.tile([P, 8], f32, tag="sdtf", name="sdtf")
                    if cnt < P:
                        nc.vector.memset(sdtf[:], 0.0)
                    nc.sync.dma_start(out=sdtf[:cnt, :], in_=sdo[t * P : t * P + cnt, :])
                    sdtb = wpool.tile([P, 8], bf16, tag="sdtb", name="sdtb")
                    nc.vector.tensor_copy(sdtb[:], sdtf[:])
                    # per-chunk SD = transpose(O_j).T @ sdt  (PE; overlaps gather)
                    psSD = ppool.tile([P, NCH * 8], f32, tag="psSD", name="psSD", bufs=1)
                    for j in range(NCH):
                        psOT = ppool.tile([P, P], bf16, tag="xtp", name="psOT", bufs=2)
                        nc.tensor.transpose(
                            out=psOT[:], in_=Oall[:, j * P : (j + 1) * P], identity=identb_sb[:]
                        )
                        OTs = wpool.tile([P, P], bf16, tag="OTs", name="OTs")
                        nc.vector.tensor_copy(OTs[:], psOT[:])
                        nc.tensor.matmul(
                            out=psSD[:, j * 8 : (j + 1) * 8], lhsT=OTs[:], rhs=sdtb[:],
                            start=True, stop=True,
                        )
                    # gather source rows [h | s_src] (bf16) from the global table
                    G = gpool.tile([P, NCH * R2], bf16, tag="G", name="G")
                    nc.gpsimd.dma_gather(
                        out_ap=G[:].rearrange("p (j r) -> p j r", r=R2),
                        in_ap=tabA[l][:],
                        idxs_ap=isrc_sb[:, EOFF[t] // 16 : EOFF[t + 1] // 16],
                        num_idxs=EPT,
                        num_idxs_reg=ept_regs[EPT],
                        elem_size=R2,
                        single_packet=False,
                    )
                    G3 = G[:].rearrange("p (j r) -> p j r", r=R2)
                    # batched e = LeakyReLU(s_src + s_dst); EXS = exp(e) (bf16)
                    EB = wpool.tile([P, NCH * 8], f32, tag="EB", name="EB")
                    nc.vector.tensor_copy(
                        EB[:].rearrange("p (j r) -> p j r", r=8), G3[:, :, HC : HC + 8]
                    )
                    nc.vector.tensor_tensor(EB[:], EB[:], psSD[:], OP.add)
                    EB2 = wpool.tile([P, NCH * 8], f32, tag="EB2", name="EB2")
                    nc.vector.tensor_scalar(EB2[:], EB[:], NEG, None, OP.mult)
                    nc.vector.tensor_tensor(EB[:], EB[:], EB2[:], OP.max)
                    EXS = wpool.tile([P, NCH * 8], bf16, tag="EXS", name="EXS")
                    nc.scalar.activation(EXS[:], EB[:], AF.Exp)
                    # per-chunk weighted scatter
                    if HC + 8 <= 512:
                        psA = ppool.tile([P, HC + 8], f32, tag="pmm", name="psA", bufs=2)
                        psB = None
                    else:
                        psA = ppool.tile([P, 512], f32, tag="pmm", name="psA", bufs=2)
                        psB = ppool.tile([P, 8], f32, tag="pmm2", name="psB", bufs=1)
                    for j in range(NCH):
                        GEX = wpool.tile([P, HC + 8], bf16, tag="GEX", name="GEX")
                        nc.vector.tensor_tensor(
                            GEX[:, :HC].rearrange("p (h c) -> p h c", h=HEADS),
                            G3[:, j, :HC].rearrange("p (h c) -> p h c", h=HEADS),
                            EXS[:, j * 8 : (j + 1) * 8].unsqueeze(2).to_broadcast((P, HEADS, C)),
                            OP.mult,
                        )
                        nc.vector.tensor_copy(GEX[:, HC : HC + 8], EXS[:, j * 8 : (j + 1) * 8])
                        if psB is None:
                            nc.tensor.matmul(
                                out=psA[:], lhsT=Oall[:, j * P : (j + 1) * P], rhs=GEX[:],
                                start=(j == 0), stop=(j == NCH - 1),
                            )
                        else:
                            nc.tensor.matmul(
                                out=psA[:], lhsT=Oall[:, j * P : (j + 1) * P], rhs=GEX[:, :512],
                                start=(j == 0), stop=(j == NCH - 1),
                            )
                            nc.tensor.matmul(
                                out=psB[:], lhsT=Oall[:, j * P : (j + 1) * P], rhs=GEX[:, 512:],
                                start=(j == 0), stop=(j == NCH - 1),
                            )
                    den = psA[:, HC : HC + 8] if psB is None else psB[:]
                    rec = wpool.tile([P, 8], f32, tag="rec", name="rec")
                    nc.vector.tensor_scalar(rec[:], den, 1e-16, None, OP.add)
                    nc.vector.reciprocal(rec[:], rec[:])
                    res = wpool.tile([P, HC], f32, tag="res", name="res")
                    nc.vector.tensor_tensor(
                        res[:].rearrange("p (h c) -> p h c", h=HEADS),
                        psA[:, :HC].rearrange("p (h c) -> p h c", h=HEADS),
                        rec[:].unsqueeze(2).to_broadcast((P, HEADS, C)),
                        OP.mult,
                    )
                    if cfg["concat"]:
                        nc.vector.tensor_tensor(res[:], res[:], bb_sb[l][:], OP.add)
                        nc.sync.dma_start(
                            out=out_dram[t * P : t * P + cnt, :], in_=res[:cnt, :]
                        )
                        if do_stat:
                            sq = wpool.tile([P, HC], f32, tag="sq", name="sq")
                            nc.scalar.square(sq[:], res[:])
                            psS1 = ppool.tile([HC, 1], f32, tag="psS", name="psS1", bufs=2)
                            nc.tensor.matmul(
                                out=psS1[:], lhsT=res[:cnt, :], rhs=ones_sb[:cnt, :],
                                start=True, stop=True,
                            )
                            nc.vector.tensor_tensor(accS[:, 0:1], accS[:, 0:1], psS1[:], OP.add)
                            psS2 = ppool.tile([HC, 1], f32, tag="psS", name="psS2", bufs=2)
                            nc.tensor.matmul(
                                out=psS2[:], lhsT=sq[:cnt, :], rhs=ones_sb[:cnt, :],
                                start=True, stop=True,
                            )
                            nc.vector.tensor_tensor(accS[:, 1:2], accS[:, 1:2], psS2[:], OP.add)
                    else:
                        red = wpool.tile([P, C], f32, tag="red", name="red")
                        nc.vector.tensor_reduce(
                            red[:],
                            res[:].rearrange("p (h c) -> p c h", h=HEADS),
                            AX,
                            OP.add,
                        )
                        nc.vector.tensor_scalar(red[:], red[:], 1.0 / HEADS, None, OP.mult)
                        nc.vector.tensor_tensor(red[:], red[:], bb_sb[l][:, :C], OP.add)
                        nc.sync.dma_start(
                            out=out_dram[t * P : t * P + cnt, :], in_=red[:cnt, :]
                        )
                if do_stat:
                    nc.sync.dma_start(out=ostat[l][:], in_=accS[:])

            # ================= full pipeline =================
            for _rep in range(repeat_k):
                tabA, gstat = fresh_shared(_rep)
                srcs = [xin, own[0], own[1], own[2]]
                outs = [own[0], own[1], own[2], out_ext]
                for l in range(4):
                    node_phase(l, srcs[l], gstat)
                    nc.gpsimd.collective_compute(
                        "AllGather",
                        mybir.AluOpType.bypass,
                        replica_groups=[list(range(M))],
                        ins=[tabL[l].opt()],
                        outs=[tabA[l].opt()],
                    )
                    edge_phase(l, tabA, outs[l])
                    if l in (0, 2):
                        nc.gpsimd.collective_compute(
                            "AllReduce",
                            mybir.AluOpType.add,
                            replica_groups=[list(range(M))],
                            ins=[ostat[l].opt()],
                            outs=[gstat[l].opt()],
                        )
    if not nc.is_finalized():
        nc.finalize()
    return nc


def _pjrt_exec(nc, in_maps, time_reps=0):
    """Mirror of bass2jax.run_bass_via_pjrt multi-core path, holding the jitted
    executable so repeated executions can be wall-timed."""
    import time as _t
    import jax
    from jax.experimental.shard_map import shard_map
    from jax.sharding import Mesh, PartitionSpec
    from concourse import bass2jax as B, mybir as mb

    B.install_neuronx_cc_hook()
    n_cores = len(in_maps)
    partition_name = nc.partition_id_tensor.name if nc.partition_id_tensor else None
    in_names, out_names, out_avals, zero_outs = [], [], [], []
    for alloc in nc.m.functions[0].allocations:
        if not isinstance(alloc, mb.MemoryLocationSet):
            continue
        name = alloc.memorylocations[0].name
        if alloc.kind == "ExternalInput":
            if name != partition_name:
                in_names.append(name)
        elif alloc.kind == "ExternalOutput":
            out_names.append(name)
            shape = tuple(alloc.tensor_shape)
            dtype = mb.dt.np(alloc.dtype)
            out_avals.append(jax.core.ShapedArray(shape, dtype))
            zero_outs.append(np.zeros(shape, dtype))
    n_params = len(in_names)
    n_outs = len(out_avals)
    in_names.extend(out_names)
    if partition_name is not None:
        in_names.append(partition_name)
    donate = tuple(range(n_params, n_params + n_outs))

    def _body(*args):
        operands = list(args)
        if partition_name is not None:
            operands.append(B.partition_id_tensor())
        outs = B._bass_exec_p.bind(
            *operands,
            out_avals=tuple(out_avals),
            in_names=tuple(in_names),
            out_names=tuple(out_names),
            lowering_input_output_aliases=(),
            sim_require_finite=True,
            sim_require_nnan=True,
            nc=nc,
        )
        return tuple(outs)

    devices = jax.devices()[:n_cores]
    mesh = Mesh(np.asarray(devices), ("core",))
    in_specs = (PartitionSpec("core"),) * (n_params + n_outs)
    out_specs = (PartitionSpec("core"),) * len(out_names)
    sharded = jax.jit(
        shard_map(_body, mesh=mesh, in_specs=in_specs, out_specs=out_specs,
                  check_rep=False),
        donate_argnums=donate, keep_unused=True,
    )
    per_core = [[np.asarray(m_[nm]) for nm in in_names[:n_params]] for m_ in in_maps]
    concat_in = [
        np.concatenate([per_core[c][i] for c in range(n_cores)], axis=0)
        for i in range(n_params)
    ]
    from jax.sharding import NamedSharding
    shard = NamedSharding(mesh, PartitionSpec("core"))
    concat_in = [jax.device_put(a, shard) for a in concat_in]
    jax.block_until_ready(concat_in)

    def once():
        cz = [jax.device_put(np.zeros((n_cores * z.shape[0], *z.shape[1:]), z.dtype), shard)
              for z in zero_outs]
        jax.block_until_ready(cz)
        t0 = _t.perf_counter()
        out_arrs = sharded(*concat_in, *cz)
        jax.block_until_ready(out_arrs)
        return _t.perf_counter() - t0, out_arrs

    _, out_arrs = once()  # compile + first run
    times = []
    for _ in range(time_reps):
        dt, out_arrs = once()
        times.append(dt)
    res = [
        {nm: np.asarray(out_arrs[i]).reshape(n_cores, *out_avals[i].shape)[c]
         for i, nm in enumerate(out_names)}
        for c in range(n_cores)
    ]
    return res, (min(times) if times else None)


def _run(inputs, trace=False, time_reps=0, repeat_k=1):
    NCHT, edata = _preprocess(np.asarray(inputs["edge_index"]))
    consts = _host_consts(inputs)
    nc = _build(NCHT, repeat_k=repeat_k)

    x = np.asarray(inputs["x"], dtype=np.float32)
    in_maps = []
    for m in range(M):
        d = dict(consts)
        d.update(edata[m])
        d["xin"] = np.ascontiguousarray(x[m * NPC : (m + 1) * NPC])
        in_maps.append(d)

    if time_reps > 0:
        results, best_s = _pjrt_exec(nc, in_maps, time_reps=time_reps)
    else:
        from concourse.bass_utils import run_bass_kernel_spmd

        res = run_bass_kernel_spmd(nc, in_maps, core_ids=list(range(M)))
        results, best_s = res.results, None
    outs = [np.asarray(results[m]["out"]) for m in range(M)]
    full = np.concatenate(outs, axis=0).astype(np.float32)
    return full, (None if best_s is None else int(best_s * 1e9))


def kernel(**inputs):
    out, _ = _run(inputs, trace=False)
    return out


# revision 12
# speedup vs baseline: 3.1744x; 2.2060x over previous
"""Distributed Bass kernel for a 4-layer GAT autoencoder on 8 TRN2 NeuronCores.

Strategy (per sharding hint): nodes sharded across 8 cores (2500/core);
edges co-located with their destination node's core, sorted by destination;
params replicated.

v3 layout: the node phase is SHARDED — each core transforms only its own
2500 rows (inputs pre-sliced per core host-side, so all node-phase DMA uses
static local offsets), then the compact bf16 per-node tables [h | s_src]
are AllGathered so the edge phase can gather any source row. s_dst stays
local (a [2500,8] buffer). Pad edges carry an out-of-range dstloc (999), so
their one-hot column is all zeros and they contribute nothing to the
scatter or the softmax denominator — no NEGBIG dummy-row machinery.

Edge phase per dst tile: one-hot O for all chunks in one DVE op; a big bf16
dma_gather of source rows; per-edge s_dst via PE transpose(O_j) + an
8-column matmul; batched LeakyReLU/exp; per-chunk h*ex one-hot scatter
matmuls accumulating numerator and denominator in PSUM. BN statistics are
per-core partial sums (closed-group matmuls + SBUF accumulate) + a [128,2]
AllReduce; there is no separate stats pass.
"""

import sys

sys.path.insert(0, "/opt/trn_rl_repo")

import numpy as np

P = 128
M = 8
N = 20000
NPC = N // M  # 2500 nodes per core
NT = (NPC + P - 1) // P  # 20 dst tiles per core
HEADS = 8
NEG = 0.2
BN_EPS = 1e-5
PADDST = 999.0  # out-of-range dst slot for pad edges -> zero one-hot column

# layer configs: Fin, C (per-head out), concat?, bn on input?, bf16 row width R2
LAYERS = [
    dict(Fin=64, C=16, concat=True, bn=False, R2=256),
    dict(Fin=128, C=32, concat=False, bn=True, R2=384),
    dict(Fin=32, C=16, concat=True, bn=False, R2=256),
    dict(Fin=128, C=64, concat=False, bn=True, R2=640),
]
OWNW = [128, 32, 128, 64]  # own[l] row widths


def _wrap16(idx):
    """Host int array -> dma_gather index layout [16, n/16] (idx[s*16+p] at [p,s])."""
    n = idx.shape[0]
    assert n % 16 == 0
    w = np.ascontiguousarray(idx.reshape(n // 16, 16).T).astype(np.int16)
    return np.ascontiguousarray(np.tile(w, (8, 1)))  # replicated for the 8 Q7 cores


def _preprocess(edge_index):
    """Partition + sort edges; per-tile chunk counts; per-core gather indices."""
    src = np.concatenate([np.asarray(edge_index[0]), np.arange(N)]).astype(np.int64)
    dst = np.concatenate([np.asarray(edge_index[1]), np.arange(N)]).astype(np.int64)

    per_core = []
    cnts = np.zeros((M, NT), dtype=np.int64)
    for m in range(M):
        sel = (dst // NPC) == m
        s, d = src[sel], dst[sel]
        dloc = d - NPC * m
        order = np.argsort(dloc, kind="stable")
        s, dloc = s[order], dloc[order]
        tiles = []
        for t in range(NT):
            tsel = (dloc // P) == t
            st, dt_ = s[tsel], dloc[tsel] - t * P
            tiles.append((st, dt_))
            cnts[m, t] = st.shape[0]
        per_core.append(tiles)

    # per-tile chunk count: max over cores, rounded up to a multiple of 4
    # (keeps every tile's idx segment 64B-aligned for the dma_gather ucode)
    nch = np.maximum(((cnts.max(axis=0) + P - 1) // P + 3) // 4 * 4, 4)
    NCHT = [int(v) for v in nch]
    EPTT = [v * P for v in NCHT]
    TOT = int(sum(EPTT))

    data = []
    for m in range(M):
        isrc = np.zeros((TOT,), dtype=np.int64)  # pad edges gather row 0
        dloc_cols = np.full((P, sum(NCHT)), PADDST, dtype=np.float32)
        eoff = 0
        coff = 0
        for t in range(NT):
            st, dt_ = per_core[m][t]
            c = st.shape[0]
            isrc[eoff : eoff + c] = st
            dl = np.full((EPTT[t],), PADDST, dtype=np.float64)
            dl[:c] = dt_
            # column coff+j, row p  = edge (t, j*128+p)
            dloc_cols[:, coff : coff + NCHT[t]] = dl.reshape(NCHT[t], P).T
            eoff += EPTT[t]
            coff += NCHT[t]
        data.append(
            dict(idx_src=_wrap16(isrc), dstloc=np.ascontiguousarray(dloc_cols))
        )
    return NCHT, data


def _host_consts(inputs):
    """Fused weights + broadcast biases + misc consts."""
    f32 = np.float32
    c = {}
    c["iotab"] = np.tile(np.arange(P, dtype=f32)[None, :], (P, 1))
    c["ident"] = np.eye(P, dtype=f32)
    c["ones"] = np.ones((P, 1), dtype=f32)

    def fuse(W, a_s, a_d):
        # WW = [W | W@blockdiag(a_src) | W@blockdiag(a_dst)]  -> [Fin, HC+16]
        H, C_ = a_s.shape
        Ws = np.einsum("fhc,hc->fh", W.reshape(-1, H, C_), a_s)
        Wd = np.einsum("fhc,hc->fh", W.reshape(-1, H, C_), a_d)
        return np.concatenate([W, Ws, Wd], axis=1).astype(f32)

    c["ww1"] = fuse(inputs["We1"], inputs["as_e1"], inputs["ad_e1"])
    c["ww2"] = fuse(inputs["We2"], inputs["as_e2"], inputs["ad_e2"])
    c["ww3"] = fuse(inputs["Wd1"], inputs["as_d1"], inputs["ad_d1"])
    c["ww4"] = fuse(inputs["Wd2"], inputs["as_d2"], inputs["ad_d2"])
    c["bb1"] = np.tile(inputs["b_e1"][None, :], (P, 1)).astype(f32)  # [128,128]
    c["bb2"] = np.tile(inputs["b_e2"][None, :], (P, 1)).astype(f32)  # [128,32]
    c["bb3"] = np.tile(inputs["b_d1"][None, :], (P, 1)).astype(f32)  # [128,128]
    c["bb4"] = np.tile(inputs["b_d2"][None, :], (P, 1)).astype(f32)  # [128,64]
    c["bn1g"] = inputs["bn1_g"].astype(f32).reshape(-1, 1)  # [128,1]
    c["bn1b"] = inputs["bn1_b"].astype(f32).reshape(-1, 1)
    c["bn2g"] = inputs["bn2_g"].astype(f32).reshape(-1, 1)
    c["bn2b"] = inputs["bn2_b"].astype(f32).reshape(-1, 1)
    return c


def _build(NCHT, repeat_k=1):
    from concourse import bacc, bass, mybir, tile

    f32 = mybir.dt.float32
    bf16 = mybir.dt.bfloat16
    i16 = mybir.dt.int16
    nc = bacc.Bacc(
        "TRN2",
        target_bir_lowering=False,
        debug=False,
        enable_asserts=False,
        num_devices=M,
    )

    EPTT = [v * P for v in NCHT]
    TOT = sum(EPTT)
    TOTC = sum(NCHT)
    EOFF = np.concatenate([[0], np.cumsum(EPTT)]).astype(int)
    COFF = np.concatenate([[0], np.cumsum(NCHT)]).astype(int)

    def din(name, shape, dtype=f32):
        return nc.dram_tensor(name, list(shape), dtype, kind="ExternalInput")

    xin = din("xin", (NPC, 64))  # per-core slice of x
    idx_src = din("idx_src", (128, TOT // 16), i16)
    dstloc = din("dstloc", (P, TOTC))
    iotab = din("iotab", (P, P))
    ident = din("ident", (P, P))
    ones = din("ones", (P, 1))
    ww = [din(f"ww{l + 1}", (LAYERS[l]["Fin"], HEADS * LAYERS[l]["C"] + 16)) for l in range(4)]
    bb = [
        din("bb1", (P, 128)),
        din("bb2", (P, 32)),
        din("bb3", (P, 128)),
        din("bb4", (P, 64)),
    ]
    bng = [None, din("bn1g", (128, 1)), None, din("bn2g", (128, 1))]
    bnb = [None, din("bn1b", (128, 1)), None, din("bn2b", (128, 1))]
    out_ext = nc.dram_tensor("out", [NPC, 64], f32, kind="ExternalOutput")

    with tile.TileContext(nc) as tc:
        with (
            tc.tile_pool(name="dram", bufs=1, space="DRAM") as dram,
            tc.tile_pool(name="const", bufs=1) as cpool,
            tc.tile_pool(name="work", bufs=3) as wpool,
            tc.tile_pool(name="gath", bufs=2) as gpool,
            tc.tile_pool(name="psum", bufs=2, space="PSUM") as ppool,
        ):
            # ---- internal DRAM (local) ----
            tabL = [
                dram.tile([NPC, LAYERS[l]["R2"]], bf16, tag=f"tabL{l}", name=f"tabL{l}")
                for l in range(4)
            ]
            sdo = dram.tile([NPC, 8], f32, tag="sdo", name="sdo")
            own = [
                dram.tile([NPC, OWNW[l]], f32, tag=f"own{l}", name=f"own{l}")
                for l in range(3)
            ]
            ostat = [
                dram.tile([128, 2], f32, tag="ostat0", name="ostat0"),
                None,
                dram.tile([128, 2], f32, tag="ostat2", name="ostat2"),
            ]

            def fresh_shared(rep):
                sfx = "" if rep == 0 else f"r{rep}"
                tabA = [
                    dram.tile([N, LAYERS[l]["R2"]], bf16, tag=f"tabA{l}{sfx}",
                              name=f"tabA{l}{sfx}", addr_space="Shared")
                    for l in range(4)
                ]
                gstat = [
                    dram.tile([128, 2], f32, tag=f"gs0{sfx}", name=f"gs0{sfx}", addr_space="Shared"),
                    None,
                    dram.tile([128, 2], f32, tag=f"gs2{sfx}", name=f"gs2{sfx}", addr_space="Shared"),
                ]
                return tabA, gstat

            # ---- consts to SBUF ----
            def load_const(ap, shape, dtype=f32, tag=None):
                t = cpool.tile(list(shape), dtype, tag=tag, name=tag)
                nc.sync.dma_start(out=t[:], in_=ap[:])
                return t

            iotaf_sb = load_const(iotab, (P, P), tag="iotaf")
            ident_sb = load_const(ident, (P, P), tag="ident")
            ones_sb = load_const(ones, (P, 1), tag="ones")
            isrc_sb = load_const(idx_src, (128, TOT // 16), i16, tag="isrc")
            dstloc_sb = load_const(dstloc, (P, TOTC), tag="dstloc")
            ww_sb = [
                load_const(ww[l], (LAYERS[l]["Fin"], HEADS * LAYERS[l]["C"] + 16), tag=f"ww{l}")
                for l in range(4)
            ]
            bb_sb = [
                load_const(bb[0], (P, 128), tag="bb0"),
                load_const(bb[1], (P, 32), tag="bb1"),
                load_const(bb[2], (P, 128), tag="bb2"),
                load_const(bb[3], (P, 64), tag="bb3"),
            ]
            bng_sb = [None, load_const(bng[1], (128, 1), tag="bng1"), None, load_const(bng[3], (128, 1), tag="bng3")]
            bnb_sb = [None, load_const(bnb[1], (128, 1), tag="bnb1"), None, load_const(bnb[3], (128, 1), tag="bnb3")]

            AX = mybir.AxisListType.X
            OP = mybir.AluOpType
            AF = mybir.ActivationFunctionType

            iotab_sb = cpool.tile([P, P], bf16, tag="iotabf", name="iotabf")
            nc.vector.tensor_copy(iotab_sb[:], iotaf_sb[:])
            identb_sb = cpool.tile([P, P], bf16, tag="identb", name="identb")
            nc.vector.tensor_copy(identb_sb[:], ident_sb[:])

            # registers holding num_idxs values for dma_gather
            ept_regs = {}
            for v in sorted(set(EPTT)):
                r = nc.alloc_registers(name=f"ept{v}")
                nc.regs_mov(r, v)
                ept_regs[v] = nc.snap(r, donate=False)

            # ============ node phase (own 2500 rows only) ============
            def node_phase(l, src_dram, gstat):
                cfg = LAYERS[l]
                Fin, C, R2 = cfg["Fin"], cfg["C"], cfg["R2"]
                HC = HEADS * C
                scale_off = None
                if cfg["bn"]:
                    sg = wpool.tile([128, 2], f32, tag="sg", name="sg")
                    nc.sync.dma_start(out=sg[:], in_=gstat[l - 1][:])
                    mu = wpool.tile([Fin, 1], f32, tag="mu", name="mu")
                    nc.vector.tensor_scalar(mu[:], sg[:, 0:1], 1.0 / N, None, OP.mult)
                    msq = wpool.tile([Fin, 1], f32, tag="msq", name="msq")
                    nc.vector.tensor_scalar(msq[:], sg[:, 1:2], 1.0 / N, None, OP.mult)
                    var = wpool.tile([Fin, 1], f32, tag="var", name="var")
                    nc.vector.tensor_tensor(var[:], mu[:], mu[:], OP.mult)
                    nc.vector.tensor_tensor(var[:], msq[:], var[:], OP.subtract)
                    nc.vector.tensor_scalar(var[:], var[:], BN_EPS, None, OP.add)
                    sdv = wpool.tile([Fin, 1], f32, tag="sdv", name="sdv")
                    nc.scalar.activation(sdv[:], var[:], AF.Sqrt)
                    rs = wpool.tile([Fin, 1], f32, tag="rs", name="rs")
                    nc.vector.reciprocal(rs[:], sdv[:])
                    bscale = wpool.tile([Fin, 1], f32, tag="bscale", name="bscale")
                    nc.vector.tensor_tensor(bscale[:], rs[:], bng_sb[l][:], OP.mult)
                    boff = wpool.tile([Fin, 1], f32, tag="boff", name="boff")
                    nc.vector.tensor_tensor(boff[:], mu[:], bscale[:], OP.mult)
                    nc.vector.tensor_tensor(boff[:], bnb_sb[l][:], boff[:], OP.subtract)
                    scale_off = (bscale, boff)

                for t in range(NT):
                    cnt = min(P, NPC - t * P)
                    xt = wpool.tile([P, Fin], f32, tag="xt", name="xt")
                    if cnt < P:
                        nc.vector.memset(xt[:], 0.0)
                    nc.sync.dma_start(out=xt[:cnt, :], in_=src_dram[t * P : t * P + cnt, :Fin])
                    xtp = ppool.tile([Fin, P], f32, tag="xtp", name="xtp", bufs=2)
                    nc.tensor.transpose(out=xtp[:], in_=xt[:], identity=ident_sb[:])
                    xts = wpool.tile([Fin, P], f32, tag="xts", name="xts")
                    if scale_off is not None:
                        nc.vector.tensor_scalar(
                            xts[:], xtp[:], scale_off[0][:], scale_off[1][:], OP.mult, OP.add
                        )
                        nc.scalar.activation(xts[:], xts[:], AF.Relu)
                    else:
                        nc.vector.tensor_copy(xts[:], xtp[:])
                    tt = wpool.tile([P, HC + 8], bf16, tag="tt", name="tt")
                    sd = wpool.tile([P, 8], f32, tag="sd", name="sd")
                    if HC + 16 <= 512:
                        hp = ppool.tile([P, HC + 16], f32, tag="pmm", name="hp", bufs=2)
                        nc.tensor.matmul(out=hp[:], lhsT=xts[:], rhs=ww_sb[l][:], start=True, stop=True)
                        nc.vector.tensor_copy(tt[:], hp[:, : HC + 8])
                        nc.vector.tensor_copy(sd[:], hp[:, HC + 8 : HC + 16])
                    else:  # L4: 528 cols -> split 512 + 16
                        hp = ppool.tile([P, 512], f32, tag="pmm", name="hp", bufs=2)
                        hp2 = ppool.tile([P, 16], f32, tag="pmm2", name="hp2", bufs=1)
                        nc.tensor.matmul(out=hp[:], lhsT=xts[:], rhs=ww_sb[l][:, :512], start=True, stop=True)
                        nc.tensor.matmul(out=hp2[:], lhsT=xts[:], rhs=ww_sb[l][:, 512:], start=True, stop=True)
                        nc.vector.tensor_copy(tt[:, :512], hp[:])
                        nc.vector.tensor_copy(tt[:, 512:520], hp2[:, 0:8])
                        nc.vector.tensor_copy(sd[:], hp2[:, 8:16])
                    nc.sync.dma_start(
                        out=tabL[l][t * P : t * P + cnt, : HC + 8], in_=tt[:cnt, :]
                    )
                    nc.sync.dma_start(out=sdo[t * P : t * P + cnt, :], in_=sd[:cnt, :])

            # ============ edge phase (own dst tiles) ============
            def edge_phase(l, tabA, out_dram):
                cfg = LAYERS[l]
                C, R2 = cfg["C"], cfg["R2"]
                HC = HEADS * C
                do_stat = l in (0, 2)
                if do_stat:
                    accS = wpool.tile([HC, 2], f32, tag="accS", name="accS")
                    nc.vector.memset(accS[:], 0.0)
                for t in range(NT):
                    NCH = NCHT[t]
                    EPT = EPTT[t]
                    cnt = min(P, NPC - t * P)
                    # one-hot O for all chunks of this tile (single DVE op);
                    # pad edges have dstloc=999 -> all-zero column
                    Oall = gpool.tile([P, NCH * P], bf16, tag="Oall", name="Oall")
                    nc.vector.tensor_tensor(
                        Oall[:].rearrange("p (j f) -> p j f", f=P),
                        iotab_sb[:].unsqueeze(1).to_broadcast((P, NCH, P)),
                        dstloc_sb[:, COFF[t] : COFF[t] + NCH].unsqueeze(2).to_broadcast((P, NCH, P)),
                        OP.is_equal,
                    )
                    # own s_dst rows for this tile (local, static offset)
                    sdtf = wpool.tile([P, 8], f32, tag="sdtf", name="sdtf")
                    if cnt < P:
                        nc.vector.memset(sdtf[:], 0.0)
                    nc.sync.dma_start(out=sdtf[:cnt, :], in_=sdo[t * P : t * P + cnt, :])
                    sdtb = wpool.tile([P, 8], bf16, tag="sdtb", name="sdtb")
                    nc.vector.tensor_copy(sdtb[:], sdtf[:])
                    # per-chunk SD = transpose(O_j).T @ sdt  (PE; overlaps gather)
                    psSD = ppool.tile([P, NCH * 8], f32, tag="psSD", name="psSD", bufs=1)
                    for j in range(NCH):
                        psOT = ppool.tile([P, P], bf16, tag="xtp", name="psOT", bufs=2)
                        nc.tensor.transpose(
                            out=psOT[:], in_=Oall[:, j * P : (j + 1) * P], identity=identb_sb[:]
                        )
                        OTs = wpool.tile([P, P], bf16, tag="OTs", name="OTs")
                        nc.scalar.activation(OTs[:], psOT[:], AF.Identity)
                        nc.tensor.matmul(
                            out=psSD[:, j * 8 : (j + 1) * 8], lhsT=OTs[:], rhs=sdtb[:],
                            start=True, stop=True,
                        )
                    # gather source rows [h | s_src] (bf16) from the global table
                    G = gpool.tile([P, NCH * R2], bf16, tag="G", name="G")
                    nc.gpsimd.dma_gather(
                        out_ap=G[:].rearrange("p (j r) -> p j r", r=R2),
                        in_ap=tabA[l][:],
                        idxs_ap=isrc_sb[:, EOFF[t] // 16 : EOFF[t + 1] // 16],
                        num_idxs=EPT,
                        num_idxs_reg=ept_regs[EPT],
                        elem_size=R2,
                        single_packet=False,
                    )
                    G3 = G[:].rearrange("p (j r) -> p j r", r=R2)
                    # batched e = LeakyReLU(s_src + s_dst); EXS = exp(e) (bf16)
                    EB = wpool.tile([P, NCH * 8], f32, tag="EB", name="EB")
                    nc.vector.tensor_copy(
                        EB[:].rearrange("p (j r) -> p j r", r=8), G3[:, :, HC : HC + 8]
                    )
                    nc.vector.tensor_tensor(EB[:], EB[:], psSD[:], OP.add)
                    EB2 = wpool.tile([P, NCH * 8], f32, tag="EB2", name="EB2")
                    nc.vector.tensor_scalar(EB2[:], EB[:], NEG, None, OP.mult)
                    nc.vector.tensor_tensor(EB[:], EB[:], EB2[:], OP.max)
                    EXS = wpool.tile([P, NCH * 8], bf16, tag="EXS", name="EXS")
                    nc.scalar.activation(EXS[:], EB[:], AF.Exp)
                    # per-chunk weighted scatter
                    if HC + 8 <= 512:
                        psA = ppool.tile([P, HC + 8], f32, tag="pmm", name="psA", bufs=2)
                        psB = None
                    else:
                        psA = ppool.tile([P, 512], f32, tag="pmm", name="psA", bufs=2)
                        psB = ppool.tile([P, 8], f32, tag="pmm2", name="psB", bufs=1)
                    for j in range(NCH):
                        GEX = wpool.tile([P, HC + 8], bf16, tag="GEX", name="GEX")
                        nc.vector.tensor_tensor(
                            GEX[:, :HC].rearrange("p (h c) -> p h c", h=HEADS),
                            G3[:, j, :HC].rearrange("p (h c) -> p h c", h=HEADS),
                            EXS[:, j * 8 : (j + 1) * 8].unsqueeze(2).to_broadcast((P, HEADS, C)),
                            OP.mult,
                        )
                        nc.gpsimd.tensor_copy(GEX[:, HC : HC + 8], EXS[:, j * 8 : (j + 1) * 8])
                        if psB is None:
                            nc.tensor.matmul(
                                out=psA[:], lhsT=Oall[:, j * P : (j + 1) * P], rhs=GEX[:],
                                start=(j == 0), stop=(j == NCH - 1),
                            )
                        else:
                            nc.tensor.matmul(
                                out=psA[:], lhsT=Oall[:, j * P : (j + 1) * P], rhs=GEX[:, :512],
                                start=(j == 0), stop=(j == NCH - 1),
                            )
                            nc.tensor.matmul(
                                out=psB[:], lhsT=Oall[:, j * P : (j + 1) * P], rhs=GEX[:, 512:],
                                start=(j == 0), stop=(j == NCH - 1),
                            )
                    den = psA[:, HC : HC + 8] if psB is None else psB[:]
                    rec = wpool.tile([P, 8], f32, tag="rec", name="rec")
                    nc.vector.tensor_scalar(rec[:], den, 1e-16, None, OP.add)
                    nc.vector.reciprocal(rec[:], rec[:])
                    res = wpool.tile([P, HC], f32, tag="res", name="res")
                    nc.vector.tensor_tensor(
                        res[:].rearrange("p (h c) -> p h c", h=HEADS),
                        psA[:, :HC].rearrange("p (h c) -> p h c", h=HEADS),
                        rec[:].unsqueeze(2).to_broadcast((P, HEADS, C)),
                        OP.mult,
                    )
                    if cfg["concat"]:
                        nc.vector.tensor_tensor(res[:], res[:], bb_sb[l][:], OP.add)
                        nc.sync.dma_start(
                            out=out_dram[t * P : t * P + cnt, :], in_=res[:cnt, :]
                        )
                        if do_stat:
                            sq = wpool.tile([P, HC], f32, tag="sq", name="sq")
                            nc.scalar.square(sq[:], res[:])
                            psS1 = ppool.tile([HC, 1], f32, tag="psS", name="psS1", bufs=2)
                            nc.tensor.matmul(
                                out=psS1[:], lhsT=res[:cnt, :], rhs=ones_sb[:cnt, :],
                                start=True, stop=True,
                            )
                            nc.vector.tensor_tensor(accS[:, 0:1], accS[:, 0:1], psS1[:], OP.add)
                            psS2 = ppool.tile([HC, 1], f32, tag="psS", name="psS2", bufs=2)
                            nc.tensor.matmul(
                                out=psS2[:], lhsT=sq[:cnt, :], rhs=ones_sb[:cnt, :],
                                start=True, stop=True,
                            )
                            nc.vector.tensor_tensor(accS[:, 1:2], accS[:, 1:2], psS2[:], OP.add)
                    else:
                        red = wpool.tile([P, C], f32, tag="red", name="red")
                        nc.vector.tensor_reduce(
                            red[:],
                            res[:].rearrange("p (h c) -> p c h", h=HEADS),
                            AX,
                            OP.add,
                        )
                        nc.vector.tensor_scalar(red[:], red[:], 1.0 / HEADS, None, OP.mult)
                        nc.vector.tensor_tensor(red[:], red[:], bb_sb[l][:, :C], OP.add)
                        nc.sync.dma_start(
                            out=out_dram[t * P : t * P + cnt, :], in_=red[:cnt, :]
                        )
                if do_stat:
                    nc.sync.dma_start(out=ostat[l][:], in_=accS[:])

            # ================= full pipeline =================
            for _rep in range(repeat_k):
                tabA, gstat = fresh_shared(_rep)
                srcs = [xin, own[0], own[1], own[2]]
                outs = [own[0], own[1], own[2], out_ext]
                for l in range(4):
                    node_phase(l, srcs[l], gstat)
                    nc.gpsimd.collective_compute(
                        "AllGather",
                        mybir.AluOpType.bypass,
                        replica_groups=[list(range(M))],
                        ins=[tabL[l].opt()],
                        outs=[tabA[l].opt()],
                    )
                    edge_phase(l, tabA, outs[l])
                    if l in (0, 2):
                        nc.gpsimd.collective_compute(
                            "AllReduce",
                            mybir.AluOpType.add,
                            replica_groups=[list(range(M))],
                            ins=[ostat[l].opt()],
                            outs=[gstat[l].opt()],
                        )
    if not nc.is_finalized():
        nc.finalize()
    return nc


def _pjrt_exec(nc, in_maps, time_reps=0):
    """Mirror of bass2jax.run_bass_via_pjrt multi-core path, holding the jitted
    executable so repeated executions can be wall-timed."""
    import time as _t
    import jax
    from jax.experimental.shard_map import shard_map
    from jax.sharding import Mesh, PartitionSpec
    from concourse import bass2jax as B, mybir as mb

    B.install_neuronx_cc_hook()
    n_cores = len(in_maps)
    partition_name = nc.partition_id_tensor.name if nc.partition_id_tensor else None
    in_names, out_names, out_avals, zero_outs = [], [], [], []
    for alloc in nc.m.functions[0].allocations:
        if not isinstance(alloc, mb.MemoryLocationSet):
            continue
        name = alloc.memorylocations[0].name
        if alloc.kind == "ExternalInput":
            if name != partition_name:
                in_names.append(name)
        elif alloc.kind == "ExternalOutput":
            out_names.append(name)
            shape = tuple(alloc.tensor_shape)
            dtype = mb.dt.np(alloc.dtype)
            out_avals.append(jax.core.ShapedArray(shape, dtype))
            zero_outs.append(np.zeros(shape, dtype))
    n_params = len(in_names)
    n_outs = len(out_avals)
    in_names.extend(out_names)
    if partition_name is not None:
        in_names.append(partition_name)
    donate = tuple(range(n_params, n_params + n_outs))

    def _body(*args):
        operands = list(args)
        if partition_name is not None:
            operands.append(B.partition_id_tensor())
        outs = B._bass_exec_p.bind(
            *operands,
            out_avals=tuple(out_avals),
            in_names=tuple(in_names),
            out_names=tuple(out_names),
            lowering_input_output_aliases=(),
            sim_require_finite=True,
            sim_require_nnan=True,
            nc=nc,
        )
        return tuple(outs)

    devices = jax.devices()[:n_cores]
    mesh = Mesh(np.asarray(devices), ("core",))
    in_specs = (PartitionSpec("core"),) * (n_params + n_outs)
    out_specs = (PartitionSpec("core"),) * len(out_names)
    sharded = jax.jit(
        shard_map(_body, mesh=mesh, in_specs=in_specs, out_specs=out_specs,
                  check_rep=False),
        donate_argnums=donate, keep_unused=True,
    )
    per_core = [[np.asarray(m_[nm]) for nm in in_names[:n_params]] for m_ in in_maps]
    concat_in = [
        np.concatenate([per_core[c][i] for c in range(n_cores)], axis=0)
        for i in range(n_params)
    ]
    from jax.sharding import NamedSharding
    shard = NamedSharding(mesh, PartitionSpec("core"))
    concat_in = [jax.device_put(a, shard) for a in concat_in]
    jax.block_until_ready(concat_in)

    def once():
        cz = [jax.device_put(np.zeros((n_cores * z.shape[0], *z.shape[1:]), z.dtype), shard)
              for z in zero_outs]
        jax.block_until_ready(cz)
        t0 = _t.perf_counter()
        out_arrs = sharded(*concat_in, *cz)
        jax.block_until_ready(out_arrs)
        return _t.perf_counter() - t0, out_arrs

    _, out_arrs = once()  # compile + first run
    times = []
    for _ in range(time_reps):
        dt, out_arrs = once()
        times.append(dt)
    res = [
        {nm: np.asarray(out_arrs[i]).reshape(n_cores, *out_avals[i].shape)[c]
         for i, nm in enumerate(out_names)}
        for c in range(n_cores)
    ]
    return res, (min(times) if times else None)


def _run(inputs, trace=False, time_reps=0, repeat_k=1):
    NCHT, edata = _preprocess(np.asarray(inputs["edge_index"]))
    consts = _host_consts(inputs)
    nc = _build(NCHT, repeat_k=repeat_k)

    x = np.asarray(inputs["x"], dtype=np.float32)
    in_maps = []
    for m in range(M):
        d = dict(consts)
        d.update(edata[m])
        d["xin"] = np.ascontiguousarray(x[m * NPC : (m + 1) * NPC])
        in_maps.append(d)

    if time_reps > 0:
        results, best_s = _pjrt_exec(nc, in_maps, time_reps=time_reps)
    else:
        from concourse.bass_utils import run_bass_kernel_spmd

        res = run_bass_kernel_spmd(nc, in_maps, core_ids=list(range(M)))
        results, best_s = res.results, None
    outs = [np.asarray(results[m]["out"]) for m in range(M)]
    full = np.concatenate(outs, axis=0).astype(np.float32)
    return full, (None if best_s is None else int(best_s * 1e9))


def kernel(**inputs):
    out, _ = _run(inputs, trace=False)
    return out
